# revision 1
# baseline (speedup 1.0000x reference)
"""MetapathAggrLayer Trainium2 kernel — v9 (DVE+ACT only; depth-3 combine).

Per node n: e_m = leakyrelu(x[m,n,:].a), w = softmax(e), out = sum_m w_m x[m,n,:].
Data-parallel over N across 8 NeuronCores; nodes-on-partitions layout.

Key TRN2 constraint discovered on HW: DVE 2-port ops (any tensor_tensor /
custom 2-input op) and GpSimd arbitrate an EXCLUSIVE lock on a shared SBUF
port pair — whichever starts first fully blocks the other for the whole
instruction. DVE is ~95% 2-port here, so GpSimd compute extends the period
1:1 and is useless: all products/adds go on DVE, the per-t scale loop on
Scalar (private port, 331ns per [128,64] op), nothing on GpSimd, and the
store is HWDGE fp32 (no SWDGE descriptor generation on GpSimd either).

Macro-tile = 4096 nodes (128 partitions x T=32), software-pipelined:

  iter i   Sync : load(i) [merged fp32 [128,(m t f)]], store(i-3)
           DVE  : scan_a(i), lrelu_a(i), scan_b(i), lrelu_b(i),
                  a01(i-3), a23(i-3), acc(i-3) [fp32 out],
                  sred(i), recip(i), w(i), t0[19:32](i), t1(i), t3(i)
           ACT  : t2[0:32](i-1), exp(i), t0[0:19](i-1)

Scores: two merged multiply+prefix-scan custom DVE ops (metapath pairs)
sharing one half-size P buffer; segment sums via boundary differences fused
with leakyrelu; softmax sum via a single strided tensor_reduce.
"""

import sys

sys.path.insert(0, "/opt/trn_rl_repo")

import numpy as np

import concourse.bacc as bacc
import concourse.mybir as mybir
from concourse import bass_utils, dve_ops
from concourse.dve_spec import Spec, Src0, Src1, C0, scan, maxx, AluOp, lower, _has_src1
from concourse.dve_uop import DveOpSpec
from concourse.tile import TileContext

ALPHA = 0.2
NMETA = 4
F = 64
N_FULL = 1_000_000
N_CORES = 8
T = 32                     # nodes per partition per macro-tile
NODES_PER_MACRO = 128 * T  # 4096
MACROS_PER_CORE = 31
NC_NODES = MACROS_PER_CORE * NODES_PER_MACRO  # 126_976
N_PAD = N_CORES * NC_NODES                    # 1_015_808
NSEG = NMETA * T           # score segments per partition per macro
NM = T * F                 # per-metapath free elems
NALL = NMETA * NM          # merged free elems
NH = NALL // 2             # half (one metapath pair)
T0_ACT = 19                # t0 chunks on Scalar; rest on Vector

_CACHE = {}


def _register_op(name, spec, subdim=False):
    if name in dve_ops._SUB_OPCODE_FOR_NAME:
        return next(o for o in dve_ops.OPS if o.name == name)
    row = dve_ops._CUSTOM_DVE_ROW_BASE + len(dve_ops.OPS)
    assert row < 0x20
    shas = {}
    for ver in ("v3", "v4"):
        s = DveOpSpec(name=name, opcode=row, uops=lower(spec, ver=ver),
                      rd1_en=_has_src1(spec))
        shas[ver] = s.sha(ver)
    op = dve_ops.DveOp(name, spec, subdim, shas)
    dve_ops.OPS.append(op)
    dve_ops.CUSTOM_DVE_SPECS[name] = spec
    dve_ops._SUB_OPCODE_FOR_NAME[name] = row
    return op


def _get_ops():
    scan_mul = _register_op(
        "MPA_SCAN_MUL",
        Spec(
            body=scan(AluOp.ADD, Src0 * Src1),
            reference=lambda in0, in1, s0, s1: np.cumsum(
                (in0.astype(np.float32) * in1.astype(np.float32)), axis=-1
            ),
        ),
    )
    ext_lrelu = _register_op(
        "MPA_EXT_LRELU",
        Spec(
            body=(lambda d: maxx(d, d * C0))(Src0 - Src1),
            reference=lambda in0, in1, s0, s1: np.maximum(in0 - in1, (in0 - in1) * s0),
        ),
    )
    return scan_mul, ext_lrelu


def _build_kernel():
    scan_mul, ext_lrelu = _get_ops()

    nc = bacc.Bacc("TRN2", target_bir_lowering=False, debug=False)
    f32 = mybir.dt.float32
    bf16 = mybir.dt.bfloat16

    x_in = nc.dram_tensor("input", (NMETA, NC_NODES, F), f32, kind="ExternalInput").ap()
    a_rep_in = nc.dram_tensor("a_rep", (128, F), f32, kind="ExternalInput").ap()
    out = nc.dram_tensor("out", (NC_NODES, F), f32, kind="ExternalOutput").ap()

    mult = mybir.AluOpType.mult
    add = mybir.AluOpType.add
    M = MACROS_PER_CORE

    with TileContext(nc) as tc:
        with tc.tile_pool(name="const", bufs=1) as cpool, \
             tc.tile_pool(name="xp", bufs=3) as xpool, \
             tc.tile_pool(name="prod", bufs=3) as prpool, \
             tc.tile_pool(name="comb", bufs=2) as copool, \
             tc.tile_pool(name="small", bufs=3) as spool:
            a_rep = cpool.tile([128, F], f32)
            nc.sync.dma_start(out=a_rep[:, :], in_=a_rep_in)
            a_bc = a_rep[:, :].rearrange("p (o f) -> p o f", o=1).broadcast_to(
                [128, NH // F, F])
            P = cpool.tile([128, NH + 1], f32)
            nc.gpsimd.memset(P[:, 0:1], 0.0)

            tiles = {}

            def wb(d, m, t0=0, t1=T):
                return d["w"][:, m * T + t0:m * T + t1].rearrange(
                    "p (t o) -> p t o", o=1).broadcast_to([128, t1 - t0, F])

            def x3(d, m, t0=0, t1=T):
                return d["xm"][:, m * NM + t0 * F:m * NM + t1 * F].rearrange(
                    "p (t f) -> p t f", f=F)

            for v in range(M + 3):
                # ---- stage A: load + scores (two half scans share P)
                if v < M:
                    lo = v * NODES_PER_MACRO
                    hi = lo + NODES_PER_MACRO
                    d = tiles[v] = {"lo": lo, "hi": hi}
                    d["xm"] = xpool.tile([128, NALL], f32, tag="x", name="xm")
                    src = x_in[:, lo:hi, :].rearrange("m (p t) f -> p m t f", p=128)
                    dst4 = d["xm"][:, :].rearrange("p (m t f) -> p m t f",
                                                   m=NMETA, f=F)
                    nc.sync.dma_start(out=dst4, in_=src)

                    d["e"] = spool.tile([128, NSEG], f32, tag="e", name="e")
                    nseg_k = NH // F
                    for k in range(2):
                        nc.vector._custom_dve(
                            scan_mul, out=P[:, 1:NH + 1],
                            in0=d["xm"][:, k * NH:(k + 1) * NH], in1=a_bc,
                        )
                        p_hi = P[:, 1:NH + 1].rearrange(
                            "p (s f) -> p s f", f=F)[:, :, F - 1:F]
                        p_lo = P[:, 0:NH].rearrange(
                            "p (s f) -> p s f", f=F)[:, :, 0:1]
                        nc.vector._custom_dve(
                            ext_lrelu,
                            out=d["e"][:, k * nseg_k:(k + 1) * nseg_k],
                            in0=p_hi, in1=p_lo, s0=ALPHA,
                        )

                # ---- ACT t2 share (iter v-1) first in the ACT queue
                if 0 <= v - 1 < M:
                    db = tiles[v - 1]
                    db["t2"] = prpool.tile([128, NM], bf16, tag="t2", name="t2")
                    for t in range(T):
                        fs = t * F
                        nc.scalar.mul(db["t2"][:, fs:fs + F],
                                      db["xm"][:, 2 * NM + fs:2 * NM + fs + F],
                                      db["w"][:, 2 * T + t:2 * T + t + 1])

                # ---- combine (iter v-3) on Vector + HWDGE store
                if 0 <= v - 3 < M:
                    dc = tiles[v - 3]
                    dc["a01"] = copool.tile([128, NM], bf16, tag="a01", name="a01")
                    dc["a23"] = copool.tile([128, NM], bf16, tag="a23", name="a23")
                    dc["acc"] = copool.tile([128, NM], f32, tag="acc", name="acc")
                    nc.vector.tensor_tensor(out=dc["a01"][:, :], in0=dc["t0"][:, :],
                                            in1=dc["t1"][:, :], op=add)
                    nc.vector.tensor_tensor(out=dc["a23"][:, :], in0=dc["t2"][:, :],
                                            in1=dc["t3"][:, :], op=add)
                    nc.vector.tensor_tensor(out=dc["acc"][:, :], in0=dc["a01"][:, :],
                                            in1=dc["a23"][:, :], op=add)
                    dst = out[dc["lo"]:dc["hi"], :].rearrange(
                        "(p t) f -> p (t f)", p=128)
                    nc.sync.dma_start(out=dst, in_=dc["acc"][:, :])
                    del tiles[v - 3]

                # ---- stage A cont.: softmax chain + DVE product shares + exp
                if v < M:
                    d = tiles[v]
                    d["u"] = spool.tile([128, NSEG], f32, tag="u", name="u")
                    nc.scalar.activation(d["u"][:, :], d["e"][:, :],
                                         mybir.ActivationFunctionType.Exp)
                    # sum over metapaths: strided reduce of [128, T, NMETA]
                    d["s"] = spool.tile([128, T], f32, tag="s", name="s")
                    u_tm = d["u"][:, :].rearrange("p (m t) -> p t m", m=NMETA)
                    nc.vector.tensor_reduce(out=d["s"][:, :], in_=u_tm,
                                            axis=mybir.AxisListType.X, op=add)
                    d["r"] = spool.tile([128, T], f32, tag="r", name="r")
                    nc.vector.reciprocal(d["r"][:, :], d["s"][:, :])
                    d["w"] = spool.tile([128, NSEG], f32, tag="w", name="w")
                    r_bc = d["r"][:, :].rearrange(
                        "p (o t) -> p o t", o=1).broadcast_to([128, NMETA, T])
                    nc.vector.tensor_tensor(
                        out=d["w"][:, :].rearrange("p (m t) -> p m t", m=NMETA),
                        in0=d["u"][:, :].rearrange("p (m t) -> p m t", m=NMETA),
                        in1=r_bc, op=mult)

                    d["t0"] = prpool.tile([128, NM], bf16, tag="t0", name="t0")
                    d["t1"] = prpool.tile([128, NM], bf16, tag="t1", name="t1")
                    d["t3"] = prpool.tile([128, NM], bf16, tag="t3", name="t3")
                    nc.vector.tensor_tensor(
                        out=d["t0"][:, T0_ACT * F:].rearrange(
                            "p (t f) -> p t f", f=F),
                        in0=x3(d, 0, T0_ACT, T), in1=wb(d, 0, T0_ACT, T), op=mult)
                    nc.vector.tensor_tensor(
                        out=d["t1"][:, :].rearrange("p (t f) -> p t f", f=F),
                        in0=x3(d, 1), in1=wb(d, 1), op=mult)
                    nc.vector.tensor_tensor(
                        out=d["t3"][:, :].rearrange("p (t f) -> p t f", f=F),
                        in0=x3(d, 3), in1=wb(d, 3), op=mult)

                # ---- ACT t0 head (iter v-1)
                if 0 <= v - 1 < M:
                    db = tiles[v - 1]
                    for t in range(T0_ACT):
                        fs = t * F
                        nc.scalar.mul(db["t0"][:, fs:fs + F], db["xm"][:, fs:fs + F],
                                      db["w"][:, t:t + 1])

    nc.compile()
    return nc


def kernel(input, a, _trace=False):
    input = np.ascontiguousarray(np.asarray(input, dtype=np.float32))
    a = np.asarray(a, dtype=np.float32).reshape(F)

    if "nc" not in _CACHE:
        _CACHE["nc"] = _build_kernel()
    nc = _CACHE["nc"]

    pad = N_PAD - input.shape[1]
    xp = np.concatenate(
        [input, np.zeros((NMETA, pad, F), np.float32)], axis=1
    ) if pad else input

    a_rep = np.tile(a[None, :], (128, 1)).astype(np.float32)

    in_maps = []
    for c in range(N_CORES):
        sl = xp[:, c * NC_NODES:(c + 1) * NC_NODES, :]
        in_maps.append({"input": np.ascontiguousarray(sl), "a_rep": a_rep})

    res = bass_utils.run_bass_kernel_spmd(
        nc, in_maps, core_ids=list(range(N_CORES)), trace=_trace
    )
    outs = [res.results[c]["out"] for c in range(N_CORES)]
    full = np.concatenate(outs, axis=0)[:N_FULL]
    if _trace:
        return full, res
    return full



# revision 5
# speedup vs baseline: 1.3653x; 1.3653x over previous
"""MetapathAggrLayer Trainium2 kernel — v10 (DVE scans + fp16 2x products
+ PE identity-matmul combine + ACT cast/evict).

Per node n: e_m = leakyrelu(x[m,n,:].a), w = softmax(e), out = sum_m w_m x[m,n,:].
Data-parallel over N across 8 NeuronCores; nodes-on-partitions layout.

Engine split per macro-tile (4096 nodes = 128 partitions x T=32):
  Sync: load xm(v) fp32 [128,(m t f)]; store osb(v-3)
  ACT : cast xm(v) -> xh fp16 (two halves), exp(v), evict psum(v-2)->osb fp32
  DVE : scan_a/lrelu_a/scan_b/lrelu_b(v)  [scores, fp32 prefix-scan trick]
        y0..y3(v-1) = xh * w2-bcast  [fp16 all-packed -> 2x_1p DVE rate]
        sred/recip/w2(v)              [softmax smalls; w2 = pair-duplicated
                                       fp16 weights so the product AP's
                                       innermost dim is packed [1,2]]
  PE  : 16 accumulating identity matmuls (v-1): psum[:,c] += I.T @ y_m[:,c]
        (fp16 moving, fp32 PSUM accumulate — replaces the DVE combine tree)

Key facts this build relies on (measured/derived from the v9 trace + cost
model source): DVE 2x_1p needs ALL operands 2-byte, innermost stride +-1,
count>=2 (broadcast stride-0 innermost disqualifies — hence w2 pair trick);
ACT scale APs are per-partition scalars only (no per-t vector scale);
matmul accumulates in PSUM with start/stop; DMA cannot touch PSUM; GpSimd
2-port ops deadlock-share an SBUF port pair with DVE 2-port ops (keep
GpSimd idle).
"""

import sys

sys.path.insert(0, "/opt/trn_rl_repo")

import numpy as np

import concourse.bacc as bacc
import concourse.mybir as mybir
from concourse import bass_utils, dve_ops
from concourse.dve_spec import Spec, Src0, Src1, C0, scan, maxx, AluOp, lower, _has_src1
from concourse.dve_uop import DveOpSpec
from concourse.tile import TileContext

ALPHA = 0.2
NMETA = 4
F = 64
N_FULL = 1_000_000
N_CORES = 8
T = 32                     # nodes per partition per macro-tile
NODES_PER_MACRO = 128 * T  # 4096
MACROS_PER_CORE = 31
NC_NODES = MACROS_PER_CORE * NODES_PER_MACRO  # 126_976
N_PAD = N_CORES * NC_NODES                    # 1_015_808
NSEG = NMETA * T           # score segments per partition per macro
NM = T * F                 # per-metapath free elems
NALL = NMETA * NM          # merged free elems
NH = NALL // 2             # half (one metapath pair)
MM_CHUNK = 512             # matmul moving free-dim limit

_CACHE = {}


def _register_op(name, spec, subdim=False):
    if name in dve_ops._SUB_OPCODE_FOR_NAME:
        return next(o for o in dve_ops.OPS if o.name == name)
    row = dve_ops._CUSTOM_DVE_ROW_BASE + len(dve_ops.OPS)
    assert row < 0x20
    shas = {}
    for ver in ("v3", "v4"):
        s = DveOpSpec(name=name, opcode=row, uops=lower(spec, ver=ver),
                      rd1_en=_has_src1(spec))
        shas[ver] = s.sha(ver)
    op = dve_ops.DveOp(name, spec, subdim, shas)
    dve_ops.OPS.append(op)
    dve_ops.CUSTOM_DVE_SPECS[name] = spec
    dve_ops._SUB_OPCODE_FOR_NAME[name] = row
    return op


def _get_ops():
    scan_mul = _register_op(
        "MPA_SCAN_MUL",
        Spec(
            body=scan(AluOp.ADD, Src0 * Src1),
            reference=lambda in0, in1, *cs: np.cumsum(
                (in0.astype(np.float32)
                 * in1.astype(np.float32).reshape(in0.shape)), axis=-1
            ),
        ),
    )
    ext_lrelu = _register_op(
        "MPA_EXT_LRELU",
        Spec(
            body=(lambda d: maxx(d, d * C0))(Src0 - Src1),
            reference=lambda in0, in1, s0=0.0, *cs: np.maximum(
                in0 - in1, (in0 - in1) * s0),
        ),
    )
    return scan_mul, ext_lrelu


def _build_kernel(macros=MACROS_PER_CORE):
    scan_mul, ext_lrelu = _get_ops()

    nc = bacc.Bacc("TRN2", target_bir_lowering=False, debug=False)
    f32 = mybir.dt.float32
    f16 = mybir.dt.float16

    nodes = macros * NODES_PER_MACRO
    x_in = nc.dram_tensor("input", (NMETA, nodes, F), f32, kind="ExternalInput").ap()
    a_rep_in = nc.dram_tensor("a_rep", (128, F), f32, kind="ExternalInput").ap()
    ident_in = nc.dram_tensor("ident", (128, 128), f16, kind="ExternalInput").ap()
    out = nc.dram_tensor("out", (nodes, F), f32, kind="ExternalOutput").ap()

    mult = mybir.AluOpType.mult
    add = mybir.AluOpType.add
    M = macros

    with TileContext(nc) as tc:
        with tc.tile_pool(name="const", bufs=1) as cpool, \
             tc.tile_pool(name="xp", bufs=3) as xpool, \
             tc.tile_pool(name="xh", bufs=2) as hpool, \
             tc.tile_pool(name="prod", bufs=2) as prpool, \
             tc.tile_pool(name="psum", bufs=2, space="PSUM") as pspool, \
             tc.tile_pool(name="osb", bufs=2) as opool, \
             tc.tile_pool(name="small", bufs=2) as spool:
            a_rep = cpool.tile([128, F], f32)
            nc.sync.dma_start(out=a_rep[:, :], in_=a_rep_in)
            ident = cpool.tile([128, 128], f16)
            nc.sync.dma_start(out=ident[:, :], in_=ident_in)
            a_bc = a_rep[:, :].rearrange("p (o f) -> p o f", o=1).broadcast_to(
                [128, NH // F, F])
            P = cpool.tile([128, NH + 1], f32)
            nc.gpsimd.memset(P[:, 0:1], 0.0)

            tiles = {}

            for v in range(M + 3):
                # ---- load (v)
                if v < M:
                    lo = v * NODES_PER_MACRO
                    hi = lo + NODES_PER_MACRO
                    d = tiles[v] = {"lo": lo, "hi": hi}
                    d["xm"] = xpool.tile([128, NALL], f32, tag="x", name="xm")
                    src = x_in[:, lo:hi, :].rearrange("m (p t) f -> p m t f", p=128)
                    dst4 = d["xm"][:, :].rearrange("p (m t f) -> p m t f",
                                                   m=NMETA, f=F)
                    nc.sync.dma_start(out=dst4, in_=src)

                # ---- store (v-3)
                if 0 <= v - 3 < M:
                    ds = tiles[v - 3]
                    dst = out[ds["lo"]:ds["hi"], :].rearrange(
                        "(p t) f -> p (t f)", p=128)
                    nc.sync.dma_start(out=dst, in_=ds["osb"][:, :])
                    del tiles[v - 3]

                # ---- ACT queue: cast(v) halves, exp(v), evict(v-2)
                if v < M:
                    d = tiles[v]
                    d["xh"] = hpool.tile([128, NALL], f16, tag="xh", name="xh")
                    nc.scalar.copy(d["xh"][:, 0:NH], d["xm"][:, 0:NH])
                    nc.scalar.copy(d["xh"][:, NH:NALL], d["xm"][:, NH:NALL])

                # ---- DVE: scores (v) — two half scans share P
                if v < M:
                    d = tiles[v]
                    d["e"] = spool.tile([128, NSEG], f32, tag="e", name="e")
                    nseg_k = NH // F
                    for k in range(2):
                        nc.vector._custom_dve(
                            scan_mul, out=P[:, 1:NH + 1],
                            in0=d["xm"][:, k * NH:(k + 1) * NH], in1=a_bc,
                        )
                        p_hi = P[:, 1:NH + 1].rearrange(
                            "p (s f) -> p s f", f=F)[:, :, F - 1:F]
                        p_lo = P[:, 0:NH].rearrange(
                            "p (s f) -> p s f", f=F)[:, :, 0:1]
                        nc.vector._custom_dve(
                            ext_lrelu,
                            out=d["e"][:, k * nseg_k:(k + 1) * nseg_k],
                            in0=p_hi, in1=p_lo, s0=ALPHA,
                        )

                # ---- ACT: exp(v)
                if v < M:
                    d = tiles[v]
                    d["u"] = spool.tile([128, NSEG], f32, tag="u", name="u")
                    nc.scalar.activation(d["u"][:, :], d["e"][:, :],
                                         mybir.ActivationFunctionType.Exp)

                # ---- DVE: products (v-1) — fp16 all-packed for 2x rate
                if 0 <= v - 1 < M:
                    db = tiles[v - 1]
                    db["y"] = prpool.tile([128, NALL], f16, tag="y", name="y")
                    for m in range(NMETA):
                        x3 = db["xh"][:, m * NM:(m + 1) * NM].rearrange(
                            "p (t f2 j) -> p t f2 j", f2=F // 2, j=2)
                        w3 = db["w2"][:, m * T * 2:(m + 1) * T * 2].rearrange(
                            "p (t o j) -> p t o j", o=1, j=2).broadcast_to(
                            [128, T, F // 2, 2])
                        y3 = db["y"][:, m * NM:(m + 1) * NM].rearrange(
                            "p (t f2 j) -> p t f2 j", f2=F // 2, j=2)
                        nc.vector.tensor_tensor(out=y3, in0=x3, in1=w3, op=mult)

                # ---- PE: combine (v-1) — psum[:,c] = sum_m y_m[:,c]
                if 0 <= v - 1 < M:
                    db = tiles[v - 1]
                    db["acc"] = pspool.tile([128, NM], f32, tag="acc", name="acc")
                    for c in range(NM // MM_CHUNK):
                        cs = c * MM_CHUNK
                        for m in range(NMETA):
                            nc.tensor.matmul(
                                db["acc"][:, cs:cs + MM_CHUNK],
                                ident[:, :],
                                db["y"][:, m * NM + cs:m * NM + cs + MM_CHUNK],
                                start=(m == 0), stop=(m == NMETA - 1),
                            )

                # ---- DVE: softmax smalls (v): s, r, w2
                if v < M:
                    d = tiles[v]
                    d["s"] = spool.tile([128, T], f32, tag="s", name="s")
                    u_tm = d["u"][:, :].rearrange("p (m t) -> p t m", m=NMETA)
                    nc.vector.tensor_reduce(out=d["s"][:, :], in_=u_tm,
                                            axis=mybir.AxisListType.X, op=add)
                    d["r"] = spool.tile([128, T], f32, tag="r", name="r")
                    nc.vector.reciprocal(d["r"][:, :], d["s"][:, :])
                    # w2[p, (m t j)] = u[p, (m t)] * r[p, t], j in {0,1}
                    # pair-duplicated so product in1 AP ends packed [1,2]
                    d["w2"] = spool.tile([128, NSEG * 2], f16, tag="w2", name="w2")
                    u_b = d["u"][:, :].rearrange(
                        "p (m t o) -> p m t o", m=NMETA, o=1).broadcast_to(
                        [128, NMETA, T, 2])
                    r_b = d["r"][:, :].rearrange(
                        "p (o t oo) -> p o t oo", o=1, oo=1).broadcast_to(
                        [128, NMETA, T, 2])
                    w2_v = d["w2"][:, :].rearrange(
                        "p (m t j) -> p m t j", m=NMETA, j=2)
                    nc.vector.tensor_tensor(out=w2_v, in0=u_b, in1=r_b, op=mult)

                # ---- ACT: evict (v-2) psum -> sbuf fp32
                if 0 <= v - 2 < M:
                    dc = tiles[v - 2]
                    dc["osb"] = opool.tile([128, NM], f32, tag="osb", name="osb")
                    nc.scalar.copy(dc["osb"][:, :], dc["acc"][:, :])

    nc.compile()
    return nc


def kernel(input, a, _trace=False):
    input = np.ascontiguousarray(np.asarray(input, dtype=np.float32))
    a = np.asarray(a, dtype=np.float32).reshape(F)

    if "nc" not in _CACHE:
        _CACHE["nc"] = _build_kernel()
    nc = _CACHE["nc"]

    pad = N_PAD - input.shape[1]
    xp = np.concatenate(
        [input, np.zeros((NMETA, pad, F), np.float32)], axis=1
    ) if pad else input

    a_rep = np.tile(a[None, :], (128, 1)).astype(np.float32)
    ident = np.eye(128, dtype=np.float16)

    in_maps = []
    for c in range(N_CORES):
        sl = xp[:, c * NC_NODES:(c + 1) * NC_NODES, :]
        in_maps.append({"input": np.ascontiguousarray(sl), "a_rep": a_rep,
                        "ident": ident})

    res = bass_utils.run_bass_kernel_spmd(
        nc, in_maps, core_ids=list(range(N_CORES)), trace=_trace
    )
    outs = [res.results[c]["out"] for c in range(N_CORES)]
    full = np.concatenate(outs, axis=0)[:N_FULL]
    if _trace:
        return full, res
    return full


# revision 8
# speedup vs baseline: 1.4657x; 1.0735x over previous
"""MetapathAggrLayer Trainium2 kernel — v10 (DVE scans + fp16 2x products
+ PE identity-matmul combine + ACT cast/evict).

Per node n: e_m = leakyrelu(x[m,n,:].a), w = softmax(e), out = sum_m w_m x[m,n,:].
Data-parallel over N across 8 NeuronCores; nodes-on-partitions layout.

Engine split per macro-tile (4096 nodes = 128 partitions x T=32):
  Sync: load xm(v) fp32 [128,(m t f)]; store osb(v-3)
  ACT : cast xm(v) -> xh fp16 (two halves), exp(v), evict psum(v-2)->osb fp32
  DVE : scan_a/lrelu_a/scan_b/lrelu_b(v)  [scores, fp32 prefix-scan trick]
        y0..y3(v-1) = xh * w2-bcast  [fp16 all-packed -> 2x_1p DVE rate]
        sred/recip/w2(v)              [softmax smalls; w2 = pair-duplicated
                                       fp16 weights so the product AP's
                                       innermost dim is packed [1,2]]
  PE  : 16 accumulating identity matmuls (v-1): psum[:,c] += I.T @ y_m[:,c]
        (fp16 moving, fp32 PSUM accumulate — replaces the DVE combine tree)

Key facts this build relies on (measured/derived from the v9 trace + cost
model source): DVE 2x_1p needs ALL operands 2-byte, innermost stride +-1,
count>=2 (broadcast stride-0 innermost disqualifies — hence w2 pair trick);
ACT scale APs are per-partition scalars only (no per-t vector scale);
matmul accumulates in PSUM with start/stop; DMA cannot touch PSUM; GpSimd
2-port ops deadlock-share an SBUF port pair with DVE 2-port ops (keep
GpSimd idle).
"""

import sys

sys.path.insert(0, "/opt/trn_rl_repo")

import numpy as np

import concourse.bacc as bacc
import concourse.mybir as mybir
from concourse import bass_utils, dve_ops
from concourse.dve_spec import Spec, Src0, Src1, C0, scan, maxx, AluOp, lower, _has_src1
from concourse.dve_uop import DveOpSpec
from concourse.tile import TileContext

ALPHA = 0.2
NMETA = 4
F = 64
N_FULL = 1_000_000
N_CORES = 8
T = 32                     # nodes per partition per macro-tile
NODES_PER_MACRO = 128 * T  # 4096
MACROS_PER_CORE = 31
NC_NODES = MACROS_PER_CORE * NODES_PER_MACRO  # 126_976
N_PAD = N_CORES * NC_NODES                    # 1_015_808
NSEG = NMETA * T           # score segments per partition per macro
NM = T * F                 # per-metapath free elems
NALL = NMETA * NM          # merged free elems
NH = NALL // 2             # half (one metapath pair)
MM_CHUNK = 512             # matmul moving free-dim limit

_CACHE = {}


def _register_op(name, spec, subdim=False):
    if name in dve_ops._SUB_OPCODE_FOR_NAME:
        return next(o for o in dve_ops.OPS if o.name == name)
    row = dve_ops._CUSTOM_DVE_ROW_BASE + len(dve_ops.OPS)
    assert row < 0x20
    shas = {}
    for ver in ("v3", "v4"):
        s = DveOpSpec(name=name, opcode=row, uops=lower(spec, ver=ver),
                      rd1_en=_has_src1(spec))
        shas[ver] = s.sha(ver)
    op = dve_ops.DveOp(name, spec, subdim, shas)
    dve_ops.OPS.append(op)
    dve_ops.CUSTOM_DVE_SPECS[name] = spec
    dve_ops._SUB_OPCODE_FOR_NAME[name] = row
    return op


def _get_ops():
    scan_mul = _register_op(
        "MPA_SCAN_MUL",
        Spec(
            body=scan(AluOp.ADD, Src0 * Src1),
            reference=lambda in0, in1, *cs: np.cumsum(
                (in0.astype(np.float32)
                 * in1.astype(np.float32).reshape(in0.shape)), axis=-1
            ),
        ),
    )
    ext_lrelu = _register_op(
        "MPA_EXT_LRELU",
        Spec(
            body=(lambda d: maxx(d, d * C0))(Src0 - Src1),
            reference=lambda in0, in1, s0=0.0, *cs: np.maximum(
                in0 - in1, (in0 - in1) * s0),
        ),
    )
    return scan_mul, ext_lrelu


def _build_kernel(macros=MACROS_PER_CORE):
    scan_mul, ext_lrelu = _get_ops()

    nc = bacc.Bacc("TRN2", target_bir_lowering=False, debug=False)
    f32 = mybir.dt.float32
    f16 = mybir.dt.float16

    nodes = macros * NODES_PER_MACRO
    x_in = nc.dram_tensor("input", (NMETA, nodes, F), f32, kind="ExternalInput").ap()
    a_rep_in = nc.dram_tensor("a_rep", (128, F), f32, kind="ExternalInput").ap()
    ident_in = nc.dram_tensor("ident", (128, 128), f16, kind="ExternalInput").ap()
    out = nc.dram_tensor("out", (nodes, F), f32, kind="ExternalOutput").ap()

    mult = mybir.AluOpType.mult
    add = mybir.AluOpType.add
    M = macros

    with TileContext(nc) as tc:
        with tc.tile_pool(name="const", bufs=1) as cpool, \
             tc.tile_pool(name="xp", bufs=3) as xpool, \
             tc.tile_pool(name="xh", bufs=2) as hpool, \
             tc.tile_pool(name="prod", bufs=2) as prpool, \
             tc.tile_pool(name="psum", bufs=2, space="PSUM") as pspool, \
             tc.tile_pool(name="osb", bufs=2) as opool, \
             tc.tile_pool(name="small", bufs=2) as spool:
            a_rep = cpool.tile([128, F], f32)
            nc.sync.dma_start(out=a_rep[:, :], in_=a_rep_in)
            ident = cpool.tile([128, 128], f16)
            nc.sync.dma_start(out=ident[:, :], in_=ident_in)
            a_bc = a_rep[:, :].rearrange("p (o f) -> p o f", o=1).broadcast_to(
                [128, NH // F, F])
            P = cpool.tile([128, NH + 1], f32)
            nc.gpsimd.memset(P[:, 0:1], 0.0)

            tiles = {}

            for v in range(M + 3):
                # ---- load (v)
                if v < M:
                    lo = v * NODES_PER_MACRO
                    hi = lo + NODES_PER_MACRO
                    d = tiles[v] = {"lo": lo, "hi": hi}
                    d["xm"] = xpool.tile([128, NALL], f32, tag="x", name="xm")
                    src = x_in[:, lo:hi, :].rearrange("m (p t) f -> p m t f", p=128)
                    dst4 = d["xm"][:, :].rearrange("p (m t f) -> p m t f",
                                                   m=NMETA, f=F)
                    nc.sync.dma_start(out=dst4, in_=src)

                # ---- store (v-3)
                if 0 <= v - 3 < M:
                    ds = tiles[v - 3]
                    dst = out[ds["lo"]:ds["hi"], :].rearrange(
                        "(p t) f -> p (t f)", p=128)
                    nc.sync.dma_start(out=dst, in_=ds["osb"][:, :])
                    del tiles[v - 3]

                # ---- ACT: evict (v-2) first (PE(v-2) finished last iter)
                if 0 <= v - 2 < M:
                    dc = tiles[v - 2]
                    dc["osb"] = opool.tile([128, NM], f32, tag="osb", name="osb")
                    nc.scalar.copy(dc["osb"][:, :], dc["acc"][:, :])

                # ---- ACT: cast(v) halves
                if v < M:
                    d = tiles[v]
                    d["xh"] = hpool.tile([128, NALL], f16, tag="xh", name="xh")
                    nc.scalar.copy(d["xh"][:, 0:NH], d["xm"][:, 0:NH])
                    nc.scalar.copy(d["xh"][:, NH:NALL], d["xm"][:, NH:NALL])

                # ---- DVE: scores (v) — two half scans share P
                if v < M:
                    d = tiles[v]
                    d["e"] = spool.tile([128, NSEG], f32, tag="e", name="e")
                    nseg_k = NH // F
                    for k in range(2):
                        nc.vector._custom_dve(
                            scan_mul, out=P[:, 1:NH + 1],
                            in0=d["xm"][:, k * NH:(k + 1) * NH], in1=a_bc,
                        )
                        p_hi = P[:, 1:NH + 1].rearrange(
                            "p (s f) -> p s f", f=F)[:, :, F - 1:F]
                        p_lo = P[:, 0:NH].rearrange(
                            "p (s f) -> p s f", f=F)[:, :, 0:1]
                        nc.vector._custom_dve(
                            ext_lrelu,
                            out=d["e"][:, k * nseg_k:(k + 1) * nseg_k],
                            in0=p_hi, in1=p_lo, s0=ALPHA,
                        )

                # ---- ACT: exp(v)
                if v < M:
                    d = tiles[v]
                    d["u"] = spool.tile([128, NSEG], f32, tag="u", name="u")
                    nc.scalar.activation(d["u"][:, :], d["e"][:, :],
                                         mybir.ActivationFunctionType.Exp)

                # ---- DVE: softmax smalls (v-1) — exp(v-1) ready since last iter
                if 0 <= v - 1 < M:
                    db = tiles[v - 1]
                    db["s"] = spool.tile([128, T], f32, tag="s", name="s")
                    u_tm = db["u"][:, :].rearrange("p (m t) -> p t m", m=NMETA)
                    nc.vector.tensor_reduce(out=db["s"][:, :], in_=u_tm,
                                            axis=mybir.AxisListType.X, op=add)
                    db["r"] = spool.tile([128, T], f32, tag="r", name="r")
                    nc.vector.reciprocal(db["r"][:, :], db["s"][:, :])
                    # w2[p, (m t j)] = u[p, (m t)] * r[p, t], j in {0,1}
                    # pair-duplicated so product in1 AP ends packed [1,2]
                    db["w2"] = spool.tile([128, NSEG * 2], f16, tag="w2", name="w2")
                    u_b = db["u"][:, :].rearrange(
                        "p (m t o) -> p m t o", m=NMETA, o=1).broadcast_to(
                        [128, NMETA, T, 2])
                    r_b = db["r"][:, :].rearrange(
                        "p (o t oo) -> p o t oo", o=1, oo=1).broadcast_to(
                        [128, NMETA, T, 2])
                    w2_v = db["w2"][:, :].rearrange(
                        "p (m t j) -> p m t j", m=NMETA, j=2)
                    nc.vector.tensor_tensor(out=w2_v, in0=u_b, in1=r_b, op=mult)

                # ---- DVE: merged product (v-1) — fp16 all-packed for 2x rate
                # w2 is contiguous over (m t), so one op covers all metapaths
                if 0 <= v - 1 < M:
                    db = tiles[v - 1]
                    db["y"] = prpool.tile([128, NALL], f16, tag="y", name="y")
                    x3 = db["xh"][:, :].rearrange(
                        "p (s f2 j) -> p s f2 j", f2=F // 2, j=2)
                    w3 = db["w2"][:, :].rearrange(
                        "p (s o j) -> p s o j", o=1, j=2).broadcast_to(
                        [128, NSEG, F // 2, 2])
                    y3 = db["y"][:, :].rearrange(
                        "p (s f2 j) -> p s f2 j", f2=F // 2, j=2)
                    nc.vector.tensor_tensor(out=y3, in0=x3, in1=w3, op=mult)

                # ---- PE: combine (v-1) — psum[:,c] = sum_m y_m[:,c]
                if 0 <= v - 1 < M:
                    db = tiles[v - 1]
                    db["acc"] = pspool.tile([128, NM], f32, tag="acc", name="acc")
                    for c in range(NM // MM_CHUNK):
                        cs = c * MM_CHUNK
                        for m in range(NMETA):
                            nc.tensor.matmul(
                                db["acc"][:, cs:cs + MM_CHUNK],
                                ident[:, :],
                                db["y"][:, m * NM + cs:m * NM + cs + MM_CHUNK],
                                start=(m == 0), stop=(m == NMETA - 1),
                            )



    nc.compile()
    return nc


def kernel(input, a, _trace=False):
    input = np.ascontiguousarray(np.asarray(input, dtype=np.float32))
    a = np.asarray(a, dtype=np.float32).reshape(F)

    if "nc" not in _CACHE:
        _CACHE["nc"] = _build_kernel()
    nc = _CACHE["nc"]

    pad = N_PAD - input.shape[1]
    xp = np.concatenate(
        [input, np.zeros((NMETA, pad, F), np.float32)], axis=1
    ) if pad else input

    a_rep = np.tile(a[None, :], (128, 1)).astype(np.float32)
    ident = np.eye(128, dtype=np.float16)

    in_maps = []
    for c in range(N_CORES):
        sl = xp[:, c * NC_NODES:(c + 1) * NC_NODES, :]
        in_maps.append({"input": np.ascontiguousarray(sl), "a_rep": a_rep,
                        "ident": ident})

    res = bass_utils.run_bass_kernel_spmd(
        nc, in_maps, core_ids=list(range(N_CORES)), trace=_trace
    )
    outs = [res.results[c]["out"] for c in range(N_CORES)]
    full = np.concatenate(outs, axis=0)[:N_FULL]
    if _trace:
        return full, res
    return full


# revision 10
# speedup vs baseline: 1.5539x; 1.0602x over previous
"""MetapathAggrLayer Trainium2 kernel — v10 (DVE scans + fp16 2x products
+ PE identity-matmul combine + ACT cast/evict).

Per node n: e_m = leakyrelu(x[m,n,:].a), w = softmax(e), out = sum_m w_m x[m,n,:].
Data-parallel over N across 8 NeuronCores; nodes-on-partitions layout.

Engine split per macro-tile (4096 nodes = 128 partitions x T=32):
  Sync: load xm(v) fp32 [128,(m t f)]; store osb(v-3)
  ACT : cast xm(v) -> xh fp16 (two halves), exp(v), evict psum(v-2)->osb fp32
  DVE : scan_a/lrelu_a/scan_b/lrelu_b(v)  [scores, fp32 prefix-scan trick]
        y0..y3(v-1) = xh * w2-bcast  [fp16 all-packed -> 2x_1p DVE rate]
        sred/recip/w2(v)              [softmax smalls; w2 = pair-duplicated
                                       fp16 weights so the product AP's
                                       innermost dim is packed [1,2]]
  PE  : 16 accumulating identity matmuls (v-1): psum[:,c] += I.T @ y_m[:,c]
        (fp16 moving, fp32 PSUM accumulate — replaces the DVE combine tree)

Key facts this build relies on (measured/derived from the v9 trace + cost
model source): DVE 2x_1p needs ALL operands 2-byte, innermost stride +-1,
count>=2 (broadcast stride-0 innermost disqualifies — hence w2 pair trick);
ACT scale APs are per-partition scalars only (no per-t vector scale);
matmul accumulates in PSUM with start/stop; DMA cannot touch PSUM; GpSimd
2-port ops deadlock-share an SBUF port pair with DVE 2-port ops (keep
GpSimd idle).
"""

import sys

sys.path.insert(0, "/opt/trn_rl_repo")

import numpy as np

import concourse.bacc as bacc
import concourse.mybir as mybir
from concourse import bass_utils, dve_ops
from concourse.dve_spec import Spec, Src0, Src1, C0, scan, maxx, AluOp, lower, _has_src1
from concourse.dve_uop import DveOpSpec
from concourse.tile import TileContext

ALPHA = 0.2
NMETA = 4
F = 64
N_FULL = 1_000_000
N_CORES = 8
T = 32                     # nodes per partition per macro-tile
NODES_PER_MACRO = 128 * T  # 4096
MACROS_PER_CORE = 31
NC_NODES = MACROS_PER_CORE * NODES_PER_MACRO  # 126_976
N_PAD = N_CORES * NC_NODES                    # 1_015_808
NSEG = NMETA * T           # score segments per partition per macro
NM = T * F                 # per-metapath free elems
NALL = NMETA * NM          # merged free elems
NH = NALL // 2             # half (one metapath pair)
MM_CHUNK = 512             # matmul moving free-dim limit

_CACHE = {}


def _register_op(name, spec, subdim=False):
    if name in dve_ops._SUB_OPCODE_FOR_NAME:
        return next(o for o in dve_ops.OPS if o.name == name)
    row = dve_ops._CUSTOM_DVE_ROW_BASE + len(dve_ops.OPS)
    assert row < 0x20
    shas = {}
    for ver in ("v3", "v4"):
        s = DveOpSpec(name=name, opcode=row, uops=lower(spec, ver=ver),
                      rd1_en=_has_src1(spec))
        shas[ver] = s.sha(ver)
    op = dve_ops.DveOp(name, spec, subdim, shas)
    dve_ops.OPS.append(op)
    dve_ops.CUSTOM_DVE_SPECS[name] = spec
    dve_ops._SUB_OPCODE_FOR_NAME[name] = row
    return op


def _get_ops():
    scan_mul = _register_op(
        "MPA_SCAN_MUL",
        Spec(
            body=scan(AluOp.ADD, Src0 * Src1),
            reference=lambda in0, in1, *cs: np.cumsum(
                (in0.astype(np.float32)
                 * in1.astype(np.float32).reshape(in0.shape)), axis=-1
            ),
        ),
    )
    ext_lrelu = _register_op(
        "MPA_EXT_LRELU",
        Spec(
            body=(lambda d: maxx(d, d * C0))(Src0 - Src1),
            reference=lambda in0, in1, s0=0.0, *cs: np.maximum(
                in0 - in1, (in0 - in1) * s0),
        ),
    )
    return scan_mul, ext_lrelu


def _build_kernel(macros=MACROS_PER_CORE):
    scan_mul, ext_lrelu = _get_ops()

    nc = bacc.Bacc("TRN2", target_bir_lowering=False, debug=False)
    f32 = mybir.dt.float32
    f16 = mybir.dt.float16

    nodes = macros * NODES_PER_MACRO
    x_in = nc.dram_tensor("input", (NMETA, nodes, F), f32, kind="ExternalInput").ap()
    a_rep_in = nc.dram_tensor("a_rep", (128, F), f32, kind="ExternalInput").ap()
    ident_in = nc.dram_tensor("ident", (128, 128), f16, kind="ExternalInput").ap()
    out = nc.dram_tensor("out", (nodes, F), f32, kind="ExternalOutput").ap()

    mult = mybir.AluOpType.mult
    add = mybir.AluOpType.add
    M = macros

    with TileContext(nc) as tc:
        with tc.tile_pool(name="const", bufs=1) as cpool, \
             tc.tile_pool(name="xp", bufs=3) as xpool, \
             tc.tile_pool(name="xh", bufs=2) as hpool, \
             tc.tile_pool(name="prod", bufs=2) as prpool, \
             tc.tile_pool(name="psum", bufs=2, space="PSUM") as pspool, \
             tc.tile_pool(name="osb", bufs=2) as opool, \
             tc.tile_pool(name="small", bufs=2) as spool:
            a_rep = cpool.tile([128, F], f32)
            nc.gpsimd.dma_start(out=a_rep[:, :], in_=a_rep_in)
            ident = cpool.tile([128, 128], f16)
            nc.gpsimd.dma_start(out=ident[:, :], in_=ident_in)
            a_bc = a_rep[:, :].rearrange("p (o f) -> p o f", o=1).broadcast_to(
                [128, NH // F, F])
            P = cpool.tile([128, NH + 1], f32)
            nc.gpsimd.memset(P[:, 0:1], 0.0)

            tiles = {}

            for v in range(M + 3):
                # ---- load (v)
                if v < M:
                    lo = v * NODES_PER_MACRO
                    hi = lo + NODES_PER_MACRO
                    d = tiles[v] = {"lo": lo, "hi": hi}
                    d["xm"] = xpool.tile([128, NALL], f32, tag="x", name="xm")
                    src = x_in[:, lo:hi, :].rearrange("m (p t) f -> p m t f", p=128)
                    dst4 = d["xm"][:, :].rearrange("p (m t f) -> p m t f",
                                                   m=NMETA, f=F)
                    nc.sync.dma_start(out=dst4, in_=src)

                # ---- store (v-3)
                if 0 <= v - 3 < M:
                    ds = tiles[v - 3]
                    dst = out[ds["lo"]:ds["hi"], :].rearrange(
                        "(p t) f -> p (t f)", p=128)
                    nc.gpsimd.dma_start(out=dst, in_=ds["osb"][:, :])
                    del tiles[v - 3]

                # ---- ACT: evict (v-2) first (PE(v-2) finished last iter)
                if 0 <= v - 2 < M:
                    dc = tiles[v - 2]
                    dc["osb"] = opool.tile([128, NM], f32, tag="osb", name="osb")
                    nc.scalar.copy(dc["osb"][:, :], dc["acc"][:, :])

                # ---- ACT: cast(v) halves
                if v < M:
                    d = tiles[v]
                    d["xh"] = hpool.tile([128, NALL], f16, tag="xh", name="xh")
                    nc.scalar.copy(d["xh"][:, 0:NH], d["xm"][:, 0:NH])
                    nc.scalar.copy(d["xh"][:, NH:NALL], d["xm"][:, NH:NALL])

                # ---- DVE: scores (v) — two half scans share P
                if v < M:
                    d = tiles[v]
                    d["e"] = spool.tile([128, NSEG], f32, tag="e", name="e")
                    nseg_k = NH // F
                    for k in range(2):
                        nc.vector._custom_dve(
                            scan_mul, out=P[:, 1:NH + 1],
                            in0=d["xm"][:, k * NH:(k + 1) * NH], in1=a_bc,
                        )
                        p_hi = P[:, 1:NH + 1].rearrange(
                            "p (s f) -> p s f", f=F)[:, :, F - 1:F]
                        p_lo = P[:, 0:NH].rearrange(
                            "p (s f) -> p s f", f=F)[:, :, 0:1]
                        nc.vector._custom_dve(
                            ext_lrelu,
                            out=d["e"][:, k * nseg_k:(k + 1) * nseg_k],
                            in0=p_hi, in1=p_lo, s0=ALPHA,
                        )

                # ---- ACT: exp(v)
                if v < M:
                    d = tiles[v]
                    d["u"] = spool.tile([128, NSEG], f32, tag="u", name="u")
                    nc.scalar.activation(d["u"][:, :], d["e"][:, :],
                                         mybir.ActivationFunctionType.Exp)

                # ---- DVE: softmax smalls (v-1) — exp(v-1) ready since last iter
                if 0 <= v - 1 < M:
                    db = tiles[v - 1]
                    db["s"] = spool.tile([128, T], f32, tag="s", name="s")
                    u_tm = db["u"][:, :].rearrange("p (m t) -> p t m", m=NMETA)
                    nc.vector.tensor_reduce(out=db["s"][:, :], in_=u_tm,
                                            axis=mybir.AxisListType.X, op=add)
                    db["r"] = spool.tile([128, T], f32, tag="r", name="r")
                    nc.vector.reciprocal(db["r"][:, :], db["s"][:, :])
                    # w2[p, (m t j)] = u[p, (m t)] * r[p, t], j in {0,1}
                    # pair-duplicated so product in1 AP ends packed [1,2]
                    db["w2"] = spool.tile([128, NSEG * 2], f16, tag="w2", name="w2")
                    u_b = db["u"][:, :].rearrange(
                        "p (m t o) -> p m t o", m=NMETA, o=1).broadcast_to(
                        [128, NMETA, T, 2])
                    r_b = db["r"][:, :].rearrange(
                        "p (o t oo) -> p o t oo", o=1, oo=1).broadcast_to(
                        [128, NMETA, T, 2])
                    w2_v = db["w2"][:, :].rearrange(
                        "p (m t j) -> p m t j", m=NMETA, j=2)
                    nc.vector.tensor_tensor(out=w2_v, in0=u_b, in1=r_b, op=mult)

                # ---- DVE: merged product (v-1) — fp16 all-packed for 2x rate
                # w2 is contiguous over (m t), so one op covers all metapaths
                if 0 <= v - 1 < M:
                    db = tiles[v - 1]
                    db["y"] = prpool.tile([128, NALL], f16, tag="y", name="y")
                    x3 = db["xh"][:, :].rearrange(
                        "p (s f2 j) -> p s f2 j", f2=F // 2, j=2)
                    w3 = db["w2"][:, :].rearrange(
                        "p (s o j) -> p s o j", o=1, j=2).broadcast_to(
                        [128, NSEG, F // 2, 2])
                    y3 = db["y"][:, :].rearrange(
                        "p (s f2 j) -> p s f2 j", f2=F // 2, j=2)
                    nc.vector.tensor_tensor(out=y3, in0=x3, in1=w3, op=mult)

                # ---- PE: combine (v-1) — psum[:,c] = sum_m y_m[:,c]
                if 0 <= v - 1 < M:
                    db = tiles[v - 1]
                    db["acc"] = pspool.tile([128, NM], f32, tag="acc", name="acc")
                    for c in range(NM // MM_CHUNK):
                        cs = c * MM_CHUNK
                        for m in range(NMETA):
                            nc.tensor.matmul(
                                db["acc"][:, cs:cs + MM_CHUNK],
                                ident[:, :],
                                db["y"][:, m * NM + cs:m * NM + cs + MM_CHUNK],
                                start=(m == 0), stop=(m == NMETA - 1),
                            )



    nc.compile()
    return nc


def kernel(input, a, _trace=False):
    input = np.ascontiguousarray(np.asarray(input, dtype=np.float32))
    a = np.asarray(a, dtype=np.float32).reshape(F)

    if "nc" not in _CACHE:
        _CACHE["nc"] = _build_kernel()
    nc = _CACHE["nc"]

    pad = N_PAD - input.shape[1]
    xp = np.concatenate(
        [input, np.zeros((NMETA, pad, F), np.float32)], axis=1
    ) if pad else input

    a_rep = np.tile(a[None, :], (128, 1)).astype(np.float32)
    ident = np.eye(128, dtype=np.float16)

    in_maps = []
    for c in range(N_CORES):
        sl = xp[:, c * NC_NODES:(c + 1) * NC_NODES, :]
        in_maps.append({"input": np.ascontiguousarray(sl), "a_rep": a_rep,
                        "ident": ident})

    res = bass_utils.run_bass_kernel_spmd(
        nc, in_maps, core_ids=list(range(N_CORES)), trace=_trace
    )
    outs = [res.results[c]["out"] for c in range(N_CORES)]
    full = np.concatenate(outs, axis=0)[:N_FULL]
    if _trace:
        return full, res
    return full


# revision 11
# speedup vs baseline: 1.5819x; 1.0180x over previous
"""MetapathAggrLayer Trainium2 kernel — v13 (DVE scans + fp16 2x products
+ PE identity-matmul combine + ACT cast/evict; split half-loads, partial
tail macro).

Per node n: e_m = leakyrelu(x[m,n,:].a), w = softmax(e), out = sum_m w_m x[m,n,:].
Data-parallel over N across 8 NeuronCores; nodes-on-partitions layout.

Engine split per macro-tile (128 partitions x T nodes; 30 macros of T=32
plus one tail macro of T=17 -> 125,056 nodes/core, 448 pad nodes total):
  Sync : load xm_a/xm_b(v) fp32 (two half DMAs so scans start earlier)
  GpSimd ring: store osb(v-3)  (separate HWDGE ring so loads never queue
               behind stores; GpSimd engine itself stays idle — its 2-port
               ops would arbitrate an exclusive SBUF port pair with DVE)
  ACT  : evict psum(v-2)->osb fp32, cast xm(v)->xh fp16, exp(v)
  DVE  : scan_a/lrelu_a/scan_b/lrelu_b(v)    [fp32 prefix-scan scores]
         sred/recip/w2(v-1)                  [exp(v-1) ready -> no stall]
         merged product y(v-1) = xh*w2-bcast [fp16 all-packed -> 2x_1p]
  PE   : accumulating identity matmuls (v-1): psum[:,c] += I.T @ y_m[:,c]

Key facts this build relies on (measured + cost-model source): DVE 2x_1p
needs ALL operands 2-byte, innermost stride +-1, count>=2 (broadcast
stride-0 innermost disqualifies — hence the pair-duplicated w2); ACT
scale APs are per-partition scalars only; matmul accumulates in PSUM with
start/stop; DMA cannot touch PSUM; fp16 products keep rel err ~7e-4.
"""

import sys

sys.path.insert(0, "/opt/trn_rl_repo")

import numpy as np

import concourse.bacc as bacc
import concourse.mybir as mybir
from concourse import bass_utils, dve_ops
from concourse.dve_spec import Spec, Src0, Src1, C0, scan, maxx, AluOp, lower, _has_src1
from concourse.dve_uop import DveOpSpec
from concourse.tile import TileContext

ALPHA = 0.2
NMETA = 4
F = 64
N_FULL = 1_000_000
N_CORES = 8
T = 32                     # nodes per partition per full macro-tile
T_TAIL = 17                # nodes per partition in the tail macro
FULL_MACROS = 30
T_LIST = [T] * FULL_MACROS + [T_TAIL]
NC_NODES = 128 * sum(T_LIST)          # 125_056
N_PAD = N_CORES * NC_NODES            # 1_000_448
NSEG = NMETA * T           # score segments per partition per full macro
NM = T * F                 # per-metapath free elems (full macro)
NALL = NMETA * NM          # merged free elems (full macro)
NH = NALL // 2             # half (one metapath pair)
MM_CHUNK = 512             # matmul moving free-dim limit

_CACHE = {}


def _register_op(name, spec, subdim=False):
    if name in dve_ops._SUB_OPCODE_FOR_NAME:
        return next(o for o in dve_ops.OPS if o.name == name)
    row = dve_ops._CUSTOM_DVE_ROW_BASE + len(dve_ops.OPS)
    assert row < 0x20
    shas = {}
    for ver in ("v3", "v4"):
        s = DveOpSpec(name=name, opcode=row, uops=lower(spec, ver=ver),
                      rd1_en=_has_src1(spec))
        shas[ver] = s.sha(ver)
    op = dve_ops.DveOp(name, spec, subdim, shas)
    dve_ops.OPS.append(op)
    dve_ops.CUSTOM_DVE_SPECS[name] = spec
    dve_ops._SUB_OPCODE_FOR_NAME[name] = row
    return op


def _get_ops():
    scan_mul = _register_op(
        "MPA_SCAN_MUL",
        Spec(
            body=scan(AluOp.ADD, Src0 * Src1),
            reference=lambda in0, in1, *cs: np.cumsum(
                (in0.astype(np.float32)
                 * in1.astype(np.float32).reshape(in0.shape)), axis=-1
            ),
        ),
    )
    ext_lrelu = _register_op(
        "MPA_EXT_LRELU",
        Spec(
            body=(lambda d: maxx(d, d * C0))(Src0 - Src1),
            reference=lambda in0, in1, s0=0.0, *cs: np.maximum(
                in0 - in1, (in0 - in1) * s0),
        ),
    )
    return scan_mul, ext_lrelu


def _build_kernel(t_list=None):
    scan_mul, ext_lrelu = _get_ops()

    nc = bacc.Bacc("TRN2", target_bir_lowering=False, debug=False)
    f32 = mybir.dt.float32
    f16 = mybir.dt.float16

    if t_list is None:
        t_list = T_LIST
    M = len(t_list)
    los = [128 * sum(t_list[:v]) for v in range(M)]
    nodes = 128 * sum(t_list)

    x_in = nc.dram_tensor("input", (NMETA, nodes, F), f32, kind="ExternalInput").ap()
    a_rep_in = nc.dram_tensor("a_rep", (128, F), f32, kind="ExternalInput").ap()
    ident_in = nc.dram_tensor("ident", (128, 128), f16, kind="ExternalInput").ap()
    out = nc.dram_tensor("out", (nodes, F), f32, kind="ExternalOutput").ap()

    mult = mybir.AluOpType.mult
    add = mybir.AluOpType.add

    with TileContext(nc) as tc:
        with tc.tile_pool(name="const", bufs=1) as cpool, \
             tc.tile_pool(name="xpa", bufs=3) as xpool_a, \
             tc.tile_pool(name="xpb", bufs=3) as xpool_b, \
             tc.tile_pool(name="xh", bufs=2) as hpool, \
             tc.tile_pool(name="prod", bufs=2) as prpool, \
             tc.tile_pool(name="psum", bufs=2, space="PSUM") as pspool, \
             tc.tile_pool(name="osb", bufs=2) as opool, \
             tc.tile_pool(name="small", bufs=2) as spool:
            a_rep = cpool.tile([128, F], f32)
            nc.gpsimd.dma_start(out=a_rep[:, :], in_=a_rep_in)
            ident = cpool.tile([128, 128], f16)
            nc.gpsimd.dma_start(out=ident[:, :], in_=ident_in)
            P = cpool.tile([128, NH + 1], f32)
            nc.gpsimd.memset(P[:, 0:1], 0.0)

            tiles = {}

            def dims(v):
                t = t_list[v]
                return t, t * F, NMETA * t  # t, nm, nseg

            for v in range(M + 3):
                # ---- load (v): two half DMAs (metapath pairs)
                if v < M:
                    t_v, nm_v, nseg_v = dims(v)
                    lo = los[v]
                    hi = lo + 128 * t_v
                    d = tiles[v] = {"lo": lo, "hi": hi, "t": t_v}
                    d["xa"] = xpool_a.tile([128, NH], f32, tag="xa", name="xa")
                    d["xb"] = xpool_b.tile([128, NH], f32, tag="xb", name="xb")
                    for k, xt in ((0, d["xa"]), (1, d["xb"])):
                        src = x_in[:, lo:hi, :][2 * k:2 * k + 2].rearrange(
                            "m (p t) f -> p m t f", p=128)
                        dst = xt[:, :2 * nm_v].rearrange(
                            "p (m t f) -> p m t f", m=2, f=F)
                        nc.sync.dma_start(out=dst, in_=src)

                # ---- store (v-3) on the gpsimd HWDGE ring
                if 0 <= v - 3 < M:
                    ds = tiles[v - 3]
                    nm_s = ds["t"] * F
                    dst = out[ds["lo"]:ds["hi"], :].rearrange(
                        "(p t) f -> p (t f)", p=128)
                    nc.gpsimd.dma_start(out=dst, in_=ds["osb"][:, :nm_s])
                    del tiles[v - 3]

                # ---- ACT: evict (v-2) first (PE(v-2) finished last iter)
                if 0 <= v - 2 < M:
                    dc = tiles[v - 2]
                    nm_c = dc["t"] * F
                    dc["osb"] = opool.tile([128, NM], f32, tag="osb", name="osb")
                    nc.scalar.copy(dc["osb"][:, :nm_c], dc["acc"][:, :nm_c])

                # ---- ACT: cast(v) halves
                if v < M:
                    d = tiles[v]
                    t_v, nm_v, nseg_v = dims(v)
                    d["xh"] = hpool.tile([128, NALL], f16, tag="xh", name="xh")
                    nc.scalar.copy(d["xh"][:, 0:2 * nm_v], d["xa"][:, :2 * nm_v])
                    nc.scalar.copy(d["xh"][:, 2 * nm_v:4 * nm_v],
                                   d["xb"][:, :2 * nm_v])

                # ---- DVE: scores (v) — two half scans share P
                if v < M:
                    d = tiles[v]
                    t_v, nm_v, nseg_v = dims(v)
                    nh_v = 2 * nm_v
                    segs_k = 2 * t_v
                    a_bc = a_rep[:, :].rearrange(
                        "p (o f) -> p o f", o=1).broadcast_to([128, segs_k, F])
                    d["e"] = spool.tile([128, NSEG], f32, tag="e", name="e")
                    for k, xt in ((0, d["xa"]), (1, d["xb"])):
                        nc.vector._custom_dve(
                            scan_mul, out=P[:, 1:nh_v + 1],
                            in0=xt[:, :nh_v], in1=a_bc,
                        )
                        p_hi = P[:, 1:nh_v + 1].rearrange(
                            "p (s f) -> p s f", f=F)[:, :, F - 1:F]
                        p_lo = P[:, 0:nh_v].rearrange(
                            "p (s f) -> p s f", f=F)[:, :, 0:1]
                        nc.vector._custom_dve(
                            ext_lrelu,
                            out=d["e"][:, k * segs_k:(k + 1) * segs_k],
                            in0=p_hi, in1=p_lo, s0=ALPHA,
                        )

                # ---- ACT: exp(v)
                if v < M:
                    d = tiles[v]
                    t_v, nm_v, nseg_v = dims(v)
                    d["u"] = spool.tile([128, NSEG], f32, tag="u", name="u")
                    nc.scalar.activation(d["u"][:, :nseg_v], d["e"][:, :nseg_v],
                                         mybir.ActivationFunctionType.Exp)

                # ---- DVE: softmax smalls (v-1) — exp(v-1) ready since last iter
                if 0 <= v - 1 < M:
                    db = tiles[v - 1]
                    t_b, nm_b, nseg_b = dims(v - 1)
                    db["s"] = spool.tile([128, T], f32, tag="s", name="s")
                    u_tm = db["u"][:, :nseg_b].rearrange(
                        "p (m t) -> p t m", m=NMETA)
                    nc.vector.tensor_reduce(out=db["s"][:, :t_b], in_=u_tm,
                                            axis=mybir.AxisListType.X, op=add)
                    db["r"] = spool.tile([128, T], f32, tag="r", name="r")
                    nc.vector.reciprocal(db["r"][:, :t_b], db["s"][:, :t_b])
                    # w2[p, (m t j)] = u[p, (m t)] * r[p, t], j in {0,1}
                    # pair-duplicated so product in1 AP ends packed [1,2]
                    db["w2"] = spool.tile([128, NSEG * 2], f16, tag="w2", name="w2")
                    u_b = db["u"][:, :nseg_b].rearrange(
                        "p (m t o) -> p m t o", m=NMETA, o=1).broadcast_to(
                        [128, NMETA, t_b, 2])
                    r_b = db["r"][:, :t_b].rearrange(
                        "p (o t oo) -> p o t oo", o=1, oo=1).broadcast_to(
                        [128, NMETA, t_b, 2])
                    w2_v = db["w2"][:, :nseg_b * 2].rearrange(
                        "p (m t j) -> p m t j", m=NMETA, j=2)
                    nc.vector.tensor_tensor(out=w2_v, in0=u_b, in1=r_b, op=mult)

                # ---- DVE: merged product (v-1) — fp16 all-packed for 2x rate
                # w2 is contiguous over (m t), so one op covers all metapaths
                if 0 <= v - 1 < M:
                    db = tiles[v - 1]
                    t_b, nm_b, nseg_b = dims(v - 1)
                    db["y"] = prpool.tile([128, NALL], f16, tag="y", name="y")
                    x3 = db["xh"][:, :4 * nm_b].rearrange(
                        "p (s f2 j) -> p s f2 j", f2=F // 2, j=2)
                    w3 = db["w2"][:, :nseg_b * 2].rearrange(
                        "p (s o j) -> p s o j", o=1, j=2).broadcast_to(
                        [128, nseg_b, F // 2, 2])
                    y3 = db["y"][:, :4 * nm_b].rearrange(
                        "p (s f2 j) -> p s f2 j", f2=F // 2, j=2)
                    nc.vector.tensor_tensor(out=y3, in0=x3, in1=w3, op=mult)

                # ---- PE: combine (v-1) — psum[:,c] = sum_m y_m[:,c]
                if 0 <= v - 1 < M:
                    db = tiles[v - 1]
                    t_b, nm_b, nseg_b = dims(v - 1)
                    db["acc"] = pspool.tile([128, NM], f32, tag="acc", name="acc")
                    cs = 0
                    while cs < nm_b:
                        ce = min(cs + MM_CHUNK, nm_b)
                        for m in range(NMETA):
                            nc.tensor.matmul(
                                db["acc"][:, cs:ce],
                                ident[:, :],
                                db["y"][:, m * nm_b + cs:m * nm_b + ce],
                                start=(m == 0), stop=(m == NMETA - 1),
                            )
                        cs = ce

    nc.compile()
    return nc


def kernel(input, a, _trace=False):
    input = np.ascontiguousarray(np.asarray(input, dtype=np.float32))
    a = np.asarray(a, dtype=np.float32).reshape(F)

    if "nc" not in _CACHE:
        _CACHE["nc"] = _build_kernel()
    nc = _CACHE["nc"]

    pad = N_PAD - input.shape[1]
    xp = np.concatenate(
        [input, np.zeros((NMETA, pad, F), np.float32)], axis=1
    ) if pad else input

    a_rep = np.tile(a[None, :], (128, 1)).astype(np.float32)
    ident = np.eye(128, dtype=np.float16)

    in_maps = []
    for c in range(N_CORES):
        sl = xp[:, c * NC_NODES:(c + 1) * NC_NODES, :]
        in_maps.append({"input": np.ascontiguousarray(sl), "a_rep": a_rep,
                        "ident": ident})

    res = bass_utils.run_bass_kernel_spmd(
        nc, in_maps, core_ids=list(range(N_CORES)), trace=_trace
    )
    outs = [res.results[c]["out"] for c in range(N_CORES)]
    full = np.concatenate(outs, axis=0)[:N_FULL]
    if _trace:
        return full, res
    return full
